# revision 7
# baseline (speedup 1.0000x reference)
"""Trainium2 Bass kernel for nn_Attention_48541720379807.

Multi-head attention (N=8 heads, H=128) with per-head K/Q projections,
softmax over projected keys, attention applied to projected keys, head
concat, and an output Linear.  B=8, L=2048, E=1024.

Sharding: pure data parallel - batch element b -> NeuronCore b.  Each core
computes its full batch slice including the output projection; the host
slices inputs and stacks outputs.  No collectives.

Per-core pipeline (PE matmuls; layouts avoid any on-device transpose of
the big input tensors - host supplies kT/qT/proj_w.T):
  A:  kxT[n] (H,L) = w_kx[n].T @ k.T   (lhsT = w slices, rhs = kT chunks)
      qxT[n] likewise; both spilled to DRAM scratch to bound SBUF.
  B:  per head, per 512-wide q block:
        scoreT[kt] (128,512) = kxT[:,kt-block].T @ qxT[:,qblk]   (PE)
        expT[kt]   = exp(score * 1/sqrt(H))            (ACT, scale fused)
        outT (H,512) += kx_nat[kc].T @ expT[kc]        (PE, accum over k)
        denom (1,512) += ones.T @ expT[kc]             (PE, interleaved)
        out_norm[:,qblk] = outT * bcast(1/denom)       (GPSIMD + DVE)
      kx_nat (k-major copy of kxT) from 16 PE transposes per head.
  C:  y (L,E) = sum_c out_norm_c[qt].T @ pwT_c + b     (PE, accum over c)

dtype mode: "f32r" (tf32 mantissa, full PE rate) or "f32" (exact fp32,
1/4 PE rate).  f32r matmul operands must be tf32-rounded by their
producer: host inputs are pre-rounded on the host; on-device
intermediates are written into f32r tiles (the copy/activation rounds).
"""

import math

import numpy as np

B, L, E, N, H = 8, 2048, 1024, 8, 128
NCORES = 8
QBLK = 512          # q block width in phase B
KCH = L // 128      # 16 k chunks / k tiles
ECH = E // 128      # 8 e chunks
SCALE = 1.0 / math.sqrt(H)

MODE = "f32r"       # "f32r" | "f32"

_CACHE = {}
_last_in_maps = None


def _round_tf32(x):
    u = np.ascontiguousarray(x, dtype=np.float32).view(np.uint32)
    add = ((u >> 13) & np.uint32(1)) + np.uint32(0x0FFF)
    return ((u + add) & np.uint32(0xFFFFE000)).view(np.float32)


def _build(mode):
    from concourse import bacc
    import concourse.mybir as mybir
    from concourse.tile import TileContext
    from concourse.masks import make_identity

    f32 = mybir.dt.float32
    mdt = mybir.dt.float32r if mode == "f32r" else f32

    nc = bacc.Bacc("TRN2", target_bir_lowering=False, debug=False,
                   num_devices=NCORES)

    kT_d = nc.dram_tensor("kT", [E, L], mdt, kind="ExternalInput")
    qT_d = nc.dram_tensor("qT", [E, L], mdt, kind="ExternalInput")
    wk_d = nc.dram_tensor("wk", [E, N * H], mdt, kind="ExternalInput")
    wq_d = nc.dram_tensor("wq", [E, N * H], mdt, kind="ExternalInput")
    pwT_d = nc.dram_tensor("pwT", [N * H, E], mdt, kind="ExternalInput")
    pb_d = nc.dram_tensor("pb", [1, E], f32, kind="ExternalInput")
    y_d = nc.dram_tensor("y", [L, E], f32, kind="ExternalOutput")
    qxT_sc = nc.dram_tensor("qxT_sc", [N * H, L], mdt)
    kxT_sc = nc.dram_tensor("kxT_sc", [N * H, L], mdt)
    on_sc = nc.dram_tensor("on_sc", [N * H, L], mdt)

    with TileContext(nc) as tc:
        with (
            tc.tile_pool(name="const", bufs=1) as const,
            tc.tile_pool(name="wsl", bufs=3) as wsl,       # (128,128) w slices
            tc.tile_pool(name="ktp", bufs=1) as ktp,       # 8x (128,1024) kT/qT half-chunks
            tc.tile_pool(name="evp", bufs=3) as evp,       # (128,1024) phase-A evict
            tc.tile_pool(name="kxth", bufs=2) as kxth,     # per-head kxT (128,2048)
            tc.tile_pool(name="onh", bufs=2) as onh,       # per-head out_norm (128,2048)
            tc.tile_pool(name="qxh", bufs=2) as qxh,       # per-head qxT (128,2048)
            tc.tile_pool(name="kxn", bufs=2) as kxn,       # per-head kx_nat (128,2048)
            tc.tile_pool(name="expp", bufs=18) as expp,    # expT (128,512)
            tc.tile_pool(name="small", bufs=2) as small,
            tc.tile_pool(name="psA", bufs=2, space="PSUM") as psA,
            tc.tile_pool(name="psS", bufs=2, space="PSUM") as psS,
            tc.tile_pool(name="psO", bufs=2, space="PSUM") as psO,
            tc.tile_pool(name="psD", bufs=2, space="PSUM") as psD,
        ):
            ident_f = const.tile([128, 128], f32)
            make_identity(nc, ident_f)
            ident = const.tile([128, 128], mdt)
            nc.vector.tensor_copy(ident[:], ident_f[:])
            ones_f = const.tile([128, 1], f32)
            nc.any.memset(ones_f[:], 1.0)
            ones = const.tile([128, 1], mdt)
            nc.vector.tensor_copy(ones[:], ones_f[:])
            pb_sb = const.tile([1, E], f32)
            nc.sync.dma_start(out=pb_sb[:], in_=pb_d[:])
            pb_bc = const.tile([128, E], f32)
            nc.gpsimd.partition_broadcast(pb_bc[:], pb_sb[:])

            # ---------------- Phase A ----------------
            def phase_a(src_d, w_d, dst_sc):
                for lh in range(2):          # l halves of 1024
                    ls = slice(lh * 1024, (lh + 1) * 1024)
                    src_tiles = []
                    for ec in range(ECH):
                        st = ktp.tile([128, 1024], mdt, tag=f"kt{ec}")
                        nc.sync.dma_start(
                            out=st[:], in_=src_d[ec * 128:(ec + 1) * 128, ls])
                        src_tiles.append(st)
                    for n in range(N):
                        w_tiles = []
                        for ec in range(ECH):
                            wt = wsl.tile([128, H], mdt, tag=f"w{ec % 4}")
                            nc.sync.dma_start(
                                out=wt[:],
                                in_=w_d[ec * 128:(ec + 1) * 128,
                                        n * H:(n + 1) * H])
                            w_tiles.append(wt)
                        ev = evp.tile([128, 1024], mdt, tag="ev")
                        for lb in range(2):
                            ps = psA.tile([128, 512], f32, tag="psA")
                            for ec in range(ECH):
                                nc.tensor.matmul(
                                    ps[:],
                                    w_tiles[ec][:],
                                    src_tiles[ec][:, lb * 512:(lb + 1) * 512],
                                    start=(ec == 0), stop=(ec == ECH - 1))
                            nc.vector.tensor_copy(
                                ev[:, lb * 512:(lb + 1) * 512], ps[:])
                        nc.sync.dma_start(
                            out=dst_sc[n * H:(n + 1) * H, ls], in_=ev[:])

            phase_a(qT_d, wq_d, qxT_sc)
            phase_a(kT_d, wk_d, kxT_sc)

            # ---------------- Phase B ----------------
            for n in range(N):
                kxT = kxth.tile([128, L], mdt, tag="kxt")
                nc.sync.dma_start(out=kxT[:], in_=kxT_sc[n * H:(n + 1) * H, :])
                qxT = qxh.tile([128, L], mdt, tag="qh")
                nc.sync.dma_start(out=qxT[:], in_=qxT_sc[n * H:(n + 1) * H, :])

                # kx_nat: (k in chunk = partition, [chunk, h] on free)
                kx_nat = kxn.tile([128, KCH * H], mdt, tag="kxn")
                for grp in range(KCH // 4):
                    pt = psD.tile([128, 512], mdt, tag="trp")
                    for j in range(4):
                        kc = grp * 4 + j
                        nc.tensor.transpose(
                            pt[:, j * 128:(j + 1) * 128],
                            kxT[:, kc * 128:(kc + 1) * 128], ident[:])
                    nc.vector.tensor_copy(
                        kx_nat[:, grp * 512:(grp + 1) * 512], pt[:])

                on = onh.tile([128, L], mdt, tag="on")
                for qb in range(L // QBLK):
                    qs = slice(qb * QBLK, (qb + 1) * QBLK)
                    exp_tiles = []
                    for kt in range(KCH):
                        ps_s = psS.tile([128, QBLK], f32, tag="psS")
                        nc.tensor.matmul(
                            ps_s[:], kxT[:, kt * 128:(kt + 1) * 128],
                            qxT[:, qs], start=True, stop=True)
                        et = expp.tile([128, QBLK], mdt, tag="expt")
                        nc.scalar.activation(
                            et[:], ps_s[:], mybir.ActivationFunctionType.Exp,
                            scale=SCALE)
                        exp_tiles.append(et)
                    ps_o = psO.tile([128, QBLK], f32, tag="psO")
                    ps_d = psD.tile([1, QBLK], f32, tag="trp")
                    for kc in range(KCH):
                        nc.tensor.matmul(
                            ps_o[:], kx_nat[:, kc * H:(kc + 1) * H],
                            exp_tiles[kc][:],
                            start=(kc == 0), stop=(kc == KCH - 1))
                        nc.tensor.matmul(
                            ps_d[:], ones[:], exp_tiles[kc][:],
                            start=(kc == 0), stop=(kc == KCH - 1))
                    d_sb = small.tile([1, QBLK], f32, tag="dsb")
                    nc.vector.tensor_copy(d_sb[:], ps_d[:])
                    d_bc = small.tile([128, QBLK], f32, tag="dbc")
                    nc.gpsimd.partition_broadcast(d_bc[:], d_sb[:])
                    d_rc = small.tile([128, QBLK], f32, tag="drc")
                    nc.vector.reciprocal(d_rc[:], d_bc[:])
                    nc.vector.tensor_mul(on[:, qs], ps_o[:], d_rc[:])
                nc.sync.dma_start(out=on_sc[n * H:(n + 1) * H, :], in_=on[:])

            # ---------------- Phase C ----------------
            pw_tiles = []
            for c in range(N):
                pwt = ktp.tile([128, E], mdt, tag=f"kt{c}")
                nc.sync.dma_start(out=pwt[:], in_=pwT_d[c * 128:(c + 1) * 128, :])
                pw_tiles.append(pwt)
            for qt in range(L // 128):
                on_tiles = []
                for c in range(N):
                    ot = wsl.tile([128, 128], mdt, tag=f"on{c % 4}")
                    nc.sync.dma_start(
                        out=ot[:],
                        in_=on_sc[c * H:(c + 1) * H, qt * 128:(qt + 1) * 128])
                    on_tiles.append(ot)
                y_sb = evp.tile([128, E], f32, tag="ysb")
                for eb in range(E // 512):
                    ps_y = psA.tile([128, 512], f32, tag="psA")
                    for c in range(N):
                        nc.tensor.matmul(
                            ps_y[:],
                            on_tiles[c][:],
                            pw_tiles[c][:, eb * 512:(eb + 1) * 512],
                            start=(c == 0), stop=(c == N - 1))
                    nc.vector.tensor_add(
                        y_sb[:, eb * 512:(eb + 1) * 512], ps_y[:],
                        pb_bc[:, eb * 512:(eb + 1) * 512])
                nc.sync.dma_start(out=y_d[qt * 128:(qt + 1) * 128, :], in_=y_sb[:])

    nc.compile()
    return nc


def _get_program(mode=MODE):
    if mode not in _CACHE:
        _CACHE[mode] = _build(mode)
    return _CACHE[mode]


def kernel(k, q, w_kx, w_qx, proj_w, proj_b, mode=MODE):
    from concourse.bass_utils import run_bass_kernel_spmd

    k = np.asarray(k, dtype=np.float32)
    q = np.asarray(q, dtype=np.float32)
    w_kx = np.asarray(w_kx, dtype=np.float32)
    w_qx = np.asarray(w_qx, dtype=np.float32)
    proj_w = np.asarray(proj_w, dtype=np.float32)
    proj_b = np.asarray(proj_b, dtype=np.float32)

    rnd = _round_tf32 if mode == "f32r" else (
        lambda x: np.ascontiguousarray(x, dtype=np.float32))
    wk = rnd(w_kx.transpose(1, 0, 2).reshape(E, N * H))   # (e, n*h)
    wq = rnd(w_qx.transpose(1, 0, 2).reshape(E, N * H))
    pwT = rnd(proj_w.T)
    pb = np.ascontiguousarray(proj_b.reshape(1, E), dtype=np.float32)

    in_maps = []
    for b in range(NCORES):
        in_maps.append({
            "kT": rnd(k[b].T),
            "qT": rnd(q[b].T),
            "wk": wk,
            "wq": wq,
            "pwT": pwT,
            "pb": pb,
        })

    global _last_in_maps
    _last_in_maps = in_maps
    nc = _get_program(mode)
    res = run_bass_kernel_spmd(nc, in_maps, list(range(NCORES)))
    out = np.stack([res.results[b]["y"] for b in range(NCORES)], axis=0)
    return out.astype(np.float32)
